# revision 11
# baseline (speedup 1.0000x reference)
"""Masked dot-product attention (B=64, S=1024, D=64) on 8 Trainium2 NeuronCores.

Strategy (per core, 8 batches, valid-length-specialized to n k-chunks/batch):
  - Two fused input DMAs per batch: head tile [Qhalf | bias | Kt chunk0]
    (1.3KB/partition, unblocks QK+exp immediately) and bulk tile
    [Kt chunks 1.. | V'] that only gates later chunks and PV.
  - S^T chunks [k=128, q=1024] = K_chunk @ Q^T on PE, D=64 contraction on
    partitions; the two 64-row strips of the PE array compute the two q-halves
    of the SAME chunk concurrently (tile_position row packing, no Q dup).
  - exp is split across TWO engines (ACT was the sole bottleneck at ~37us):
      * ACT chunks: exp via ACT, per-partition bias 0/-30000 folds the
        valid_lens mask into the softmax; 1/sqrt(D) folded into the ACT scale.
      * DVE chunks (fully-valid-for-the-whole-deal-group only): dual-offset
        Schraudolph. One tensor_scalar computes B1 = rint(s*C1 + C2) as int16
        (the f16 exponent-bias add folds into the float affine BEFORE the
        rounding); bitcast f16 gives R1 ~ c*2^y. A second int16 op B2 = B1+512
        gives R2 ~ c*sqrt(2)*2^y. R1+R2 averages the two half-period-offset
        linear-mantissa approximations, cancelling the dominant ripple
        harmonic: +0.6/-1.5% max element error (vs +-4% single), ~2.5e-3
        end-to-end. The add R1+R2 happens for free in PSUM: both are fed as
        separate lhsT slices to the PV accumulation (PE has slack).
  - P @ [V | 1]: P^T slices stationary; column 64 of the accumulator is the
    softmax denominator. normalize = reciprocal + one broadcast
    scalar_tensor_tensor per 4-q-tile half -> fp16 out, upcast on host.
Host does layout prep only (transpose/cast/pack/shard) - all FLOPs on device.
"""

import contextlib

import numpy as np

import concourse.bass as bass  # noqa: F401
import concourse.bacc as bacc
import concourse.mybir as mybir
import concourse.tile as tile
from concourse.bass_utils import run_bass_kernel_spmd

B, S, D = 64, 1024, 64
NCORES = 8
BPC = B // NCORES          # batches per core
NCH = S // 128             # k chunks of 128
NQT = S // 128             # q tiles of 128
F16 = mybir.dt.float16
F32 = mybir.dt.float32
I16 = mybir.dt.int16

# fused input row layout (f16 elements per partition):
#   [0:512)                q half (strip 0: q 0-511, strip 1: q 512-1023)
#   [512 : 512+n)          exp bias per chunk (0 / -30000, f16)
#   [512+n : 512+n+128)    K^T chunk 0
ROW = 512 + 195 * NCH  # legacy single-row length, kept for bench scripts
AROW = 512 + NCH + 128        # [qt | bias(NCH) | kt chunk0]
BROW = 128 * (NCH - 1) + 66 * NCH  # [kt chunks 1.. | vv]

# Dual-offset Schraudolph constants (DVE exp path).
#   B1 = rint(s_raw*SCHR_C1 + SCHR_C2) int16; R1 = bitcast_f16(B1)
#   B2 = B1 + 512;                           R2 = bitcast_f16(B2)
#   R1 + R2 ~ exp(s_raw/8), unbiased (c = 0.398020883 calibrated so the
#   mean ratio over a uniform mantissa phase is exactly 1).
SCHR_C1 = 184.664965234          # 1024 * log2(e) / 8
SCHR_C2 = 13999.018016           # 1024 * (15 + log2(0.398020883))

# Total DVE-offloaded chunks per core (balances ACT ~850ns/chunk vs DVE
# ~1.5us/chunk + ~8.5us finish). Tuned on HW fast-mode observations.
DVE_TARGET = 9

# normalize flavor: one broadcast scalar_tensor_tensor per half (True) vs
# four tensor_scalar_mul per half (False)
NORM_STT = True

# emission order: pace PV behind one unit + emit finish(prev) as soon as
# PV(prev) completes (True), vs baseline order (False). A/B on HW showed
# no benefit; keep the baseline-proven order.
REORDER = False

_NC_CACHE = {}


def _parse_slot(sc):
    if isinstance(sc, tuple):
        if len(sc) == 3:
            n, nz, nd = sc
        else:
            n, nz = sc
            nd = 0
    else:
        n, nz, nd = sc, sc, 0
    n = max(1, min(NCH, n))
    nz = max(1, min(n, nz))
    nd = max(0, min(nd, nz - 1))
    return n, nz, nd


def _interleave(acts, dves):
    # merge the two chunk lists evenly, dve first (chunk 0's kt rides the
    # head tile, so its QK can start before the bulk DMA lands)
    out = []
    na, nd = len(acts), len(dves)
    ia = idv = 0
    for _ in range(na + nd):
        if idv < nd and (ia >= na or idv * na <= ia * nd):
            out.append(("dve", dves[idv]))
            idv += 1
        else:
            out.append(("act", acts[ia]))
            ia += 1
    return out


def _build_nc(loop_reps=None, slot_counts=(NCH,) * BPC, ablate=frozenset()):
    # slot_counts entries: n | (n, nz) | (n, nz, nd). Chunks 0..nz-2 are
    # fully valid for EVERY batch dealt into that slot (group min need);
    # chunks 0..nd-1 of those run the DVE dual-Schraudolph exp instead of
    # the ACT exp. The rest always read their per-chunk bias vector on ACT.
    parsed = [_parse_slot(sc) for sc in slot_counts]
    # pm-ring sizing: tiles of slot s (n + nd of them) live through slot s+1
    tiles_per_slot = [n + nd for (n, nz, nd) in parsed]
    max_adj = max(
        (tiles_per_slot[i] + tiles_per_slot[i + 1]
         for i in range(len(tiles_per_slot) - 1)),
        default=tiles_per_slot[0] if tiles_per_slot else 2,
    )
    pm_bufs = min(32, max_adj + 2)

    nc = bacc.Bacc(None, target_bir_lowering=False)
    inpa = nc.dram_tensor("inpa", [BPC, 128, AROW], F16, kind="ExternalInput")
    inpb = nc.dram_tensor("inpb", [BPC, 128, BROW], F16, kind="ExternalInput")
    ot = nc.dram_tensor("ot", [BPC, 128, NQT, D], F16, kind="ExternalOutput")

    with tile.TileContext(nc) as tc:
        with (
            tc.tile_pool(name="inpool", bufs=3) as inpool,
            tc.tile_pool(name="ppool", bufs=pm_bufs) as ppool,
            tc.tile_pool(name="outpool", bufs=2) as outpool,
            tc.tile_pool(name="rpool", bufs=4) as rpool,
            tc.tile_pool(name="spool", bufs=3, space="PSUM") as spool,
            tc.tile_pool(name="accpool", bufs=1, space="PSUM") as accpool,
            tc.For_i(0, loop_reps, 1) if loop_reps else contextlib.nullcontext(),
        ):
            def emit_pv_block(prev, t):
                # one q-tile's full accumulation chain for the previous slot
                b_p, n_p, tb_p, ents_p, acc0_p, acc1_p = prev
                vo_p = 128 * (n_p - 1)
                acc = acc0_p if t < 4 else acc1_p
                total = sum(len(e) for e in ents_p)
                i = 0
                for c in range(n_p):
                    rhs = tb_p[:, vo_p + 66 * c: vo_p + 66 * c + 65]
                    for pm_t, off in ents_p[c]:
                        nc.tensor.matmul(
                            acc[:, t % 4, :],
                            lhsT=pm_t[:, off + t * 128:off + (t + 1) * 128],
                            rhs=rhs,
                            start=(i == 0), stop=(i == total - 1),
                        )
                        i += 1

            def emit_finish_half(prev, half, osb):
                # normalize + store one 4-q-tile half of the previous slot
                b_p, n_p, tb_p, ents_p, acc0_p, acc1_p = prev
                acc = acc0_p if half == 0 else acc1_p
                # denominators are bounded away from 0 (every batch has >=1
                # valid key; DVE chunks are fully valid), so no epsilon guard
                r = rpool.tile([128, 4], F32, tag="r", name="r")
                nc.vector.reciprocal(r, acc[:, :, D])
                if NORM_STT:
                    nc.vector.scalar_tensor_tensor(
                        out=osb[:, half * 4:(half + 1) * 4, :],
                        in0=acc[:, 0:4, 0:D],
                        in1=r.unsqueeze(2).broadcast_to([128, 4, D]),
                        scalar=1.0,
                        op0=mybir.AluOpType.mult, op1=mybir.AluOpType.mult,
                    )
                else:
                    for t4 in range(4):
                        t = half * 4 + t4
                        nc.vector.tensor_scalar_mul(
                            out=osb[:, t, :],
                            in0=acc[:, t4, 0:D],
                            scalar1=r[:, t4:t4 + 1],
                        )
                nc.sync.dma_start(
                    out=ot.ap()[b_p][:, half * 4:(half + 1) * 4, :],
                    in_=osb[:, half * 4:(half + 1) * 4, :],
                )

            def emit_finish(prev):
                osb = outpool.tile([128, NQT, D], F16, name="osb")
                emit_finish_half(prev, 0, osb)
                emit_finish_half(prev, 1, osb)

            # tiny dummy exp: pulls the one-time ~2.7us ACT table load to
            # t=0 so it overlaps the first input DMA instead of serializing
            # before the first real exp
            warm = rpool.tile([128, 1], F32, tag="warm", name="warm")
            nc.vector.memset(warm, 0.0)
            nc.scalar.activation(
                out=warm, in_=warm, func=mybir.ActivationFunctionType.Exp
            )

            prev = None
            for b in range(BPC):
                n, nz, nd = parsed[b]
                ua = 512 + n + 128
                ub = 128 * (n - 1) + 66 * n

                ta = inpool.tile([128, AROW], F16, tag="ta", name="ta")
                nc.sync.dma_start(out=ta[:, :ua], in_=inpa.ap()[b][:, :ua])
                tb = inpool.tile([128, BROW], F16, tag="tb", name="tb")
                nc.sync.dma_start(out=tb[:, :ub], in_=inpb.ap()[b][:, :ub])
                qt = ta[:, 0:512]

                acc0 = accpool.tile([128, 4, D + 1], F32, tag="acc0")
                acc1 = accpool.tile([128, 4, D + 1], F32, tag="acc1")

                units = _interleave(list(range(nd, n)), list(range(nd)))

                # interleave this slot's QK+exp with the previous slot's PV so
                # the in-order PE queue never parks ACT behind a PV burst
                ents = [None] * n
                nu = len(units)
                m = max(nu, NQT if prev else 0)
                pv_done = 0
                fin_done = False
                for i in range(m):
                    if prev is not None and "pv" not in ablate:
                        # pace on i (not i+1): the first unit of this slot is
                        # emitted before the first PV block, so the PE never
                        # stalls on finish(prev)'s acc-ring WAR at the slot
                        # boundary
                        ii = i if REORDER else i + 1
                        pv_goal = min(NQT, (NQT * ii + m - 1) // m)
                        while pv_done < pv_goal:
                            emit_pv_block(prev, pv_done)
                            pv_done += 1
                        if REORDER and pv_done == NQT and not fin_done:
                            # emit finish as soon as PV(prev) is complete so
                            # its DVE ops queue ahead of this slot's exp tail
                            emit_finish(prev)
                            fin_done = True
                    if i < nu:
                        kind, c = units[i]
                        st = spool.tile([128, S], F32, tag="st", name="st")
                        if "qk" not in ablate:
                            kt = (
                                ta[:, 512 + n:512 + n + 128] if c == 0
                                else tb[:, 128 * (c - 1):128 * c]
                            )
                            nc.tensor.matmul(
                                st[:, 0:512],
                                lhsT=kt[0:64, :], rhs=qt[0:64, :],
                                start=True, stop=True,
                            )
                            nc.tensor.matmul(
                                st[:, 512:1024],
                                lhsT=kt[64:128, :], rhs=qt[64:128, :],
                                start=True, stop=True,
                            )
                        if "exp" not in ablate:
                            if kind == "act":
                                pm = ppool.tile([128, S], F16, tag="pm",
                                                name="pm")
                                bias = ta[:, 512 + c:512 + c + 1]
                                nc.scalar.activation(
                                    out=pm, in_=st,
                                    func=mybir.ActivationFunctionType.Exp,
                                    bias=bias, scale=0.125,
                                )
                                ents[c] = [(pm, 0)]
                            else:
                                b1 = ppool.tile([128, S], I16, tag="pm",
                                                name="b1")
                                nc.vector.tensor_scalar(
                                    out=b1, in0=st,
                                    scalar1=SCHR_C1, scalar2=SCHR_C2,
                                    op0=mybir.AluOpType.mult,
                                    op1=mybir.AluOpType.add,
                                )
                                b2 = ppool.tile([128, S], I16, tag="pm",
                                                name="b2")
                                nc.vector.tensor_scalar_add(
                                    out=b2, in0=b1, scalar1=512
                                )
                                ents[c] = [(b1.bitcast(F16), 0),
                                           (b2.bitcast(F16), 0)]
                        else:
                            ents[c] = [(tb[:, 0:S].bitcast(F16), 0)]
                if prev is not None and "pv" not in ablate:
                    while pv_done < NQT:
                        emit_pv_block(prev, pv_done)
                        pv_done += 1
                if prev is not None:
                    if "pv" not in ablate:
                        if not fin_done:
                            emit_finish(prev)
                    else:
                        b_p, n_p, tb_p, ents_p = prev[:4]
                        src = ents_p[-1][0][0]
                        nc.sync.dma_start(
                            out=ot.ap()[b_p],
                            in_=src[:, 0:NQT * D].rearrange(
                                "p (t d) -> p t d", d=D
                            ),
                        )
                prev = (b, n, tb, ents, acc0, acc1)

            # drain the last slot: finish+store half 0 while half 1's PV runs
            if "pv" not in ablate:
                osb = outpool.tile([128, NQT, D], F16, name="osb")
                for t in range(NQT):
                    emit_pv_block(prev, t)
                    if t == 3:
                        emit_finish_half(prev, 0, osb)
                emit_finish_half(prev, 1, osb)
            else:
                b_p, n_p, tin_p, ents_p = prev[:4]
                src = ents_p[-1][0][0]
                nc.sync.dma_start(
                    out=ot.ap()[b_p],
                    in_=src[:, 0:NQT * D].rearrange("p (t d) -> p t d", d=D),
                )

    nc.compile()
    return nc


def _get_nc(slot_counts=(NCH,) * BPC):
    key = tuple(slot_counts)
    if key not in _NC_CACHE:
        _NC_CACHE[key] = _build_nc(slot_counts=key)
    return _NC_CACHE[key]


def _host_prep(queries, keys, values, valid_lens):
    queries = np.asarray(queries, dtype=np.float32)
    keys = np.asarray(keys, dtype=np.float32)
    values = np.asarray(values, dtype=np.float32)
    lens = np.asarray(valid_lens).astype(np.int64)

    q16 = queries.astype(np.float16)
    k16 = keys.astype(np.float16)
    v16 = values.astype(np.float16)

    # q halves packed into the two PE row strips: [B, 128, 512]
    qh = q16.transpose(0, 2, 1).reshape(B, 64, 2, 512)
    qh = np.ascontiguousarray(qh.transpose(0, 2, 1, 3)).reshape(B, 128, 512)

    # K^T chunks duplicated into both strips: [B, 128, NCH, 128]
    kt4 = k16.transpose(0, 2, 1).reshape(B, 64, NCH, 128)
    ktd = np.concatenate([kt4, kt4], axis=1)

    # V with ones column (pad to 66): [B, 128, NCH, 66]
    vp = np.zeros((B, 128, NCH, D + 2), np.float16)
    vp[:, :, :, :D] = v16.reshape(B, NCH, 128, D).transpose(0, 2, 1, 3)
    vp[:, :, :, D] = np.float16(1.0)

    # exp bias: 0 where k position valid, -30000 where masked: [B, 128, NCH]
    kpos = np.arange(S).reshape(NCH, 128).T  # [128, NCH] -> k = c*128 + p
    bia = np.where(
        kpos[None] < lens[:, None, None], np.float16(0.0), np.float16(-30000.0)
    ).astype(np.float16)

    # Length specialization: batch i needs ceil(L_i/128) k-chunks (min 1).
    # Sort by need, deal round-robin -> every core's slot s holds batches of
    # (near-)equal need; slot count = max within the deal group, so all cores
    # run the identical compiled program, perfectly balanced.
    need = np.maximum(1, -(-lens // 128)).astype(np.int64)
    order = np.argsort(need, kind="stable")
    gmax = [int(need[order[g * NCORES:(g + 1) * NCORES]].max()) for g in range(BPC)]
    gmin = [int(need[order[g * NCORES:(g + 1) * NCORES]].min()) for g in range(BPC)]
    perm = list(range(BPC - 1, -1, -1))  # descending: smallest slot last = tiny drain tail
    # distribute DVE_TARGET chunks across slots proportional to eligibility
    elig = [max(0, gmin[p] - 1) for p in perm]
    tot_el = sum(elig)
    tgt = min(DVE_TARGET, tot_el)
    nd = [0] * BPC
    if tot_el > 0 and tgt > 0:
        nd = [min(e, (e * tgt) // tot_el) for e in elig]
        rem = tgt - sum(nd)
        frac = sorted(
            range(BPC), key=lambda s: (elig[s] - nd[s]), reverse=True
        )
        for s in frac:
            if rem <= 0:
                break
            if nd[s] < elig[s]:
                nd[s] += 1
                rem -= 1
    slot_counts = tuple(
        (gmax[p], gmin[p], nd[si]) for si, p in enumerate(perm)
    )

    in_maps = []
    for c in range(NCORES):
        fa = np.zeros((BPC, 128, AROW), np.float16)
        fb = np.zeros((BPC, 128, BROW), np.float16)
        for s in range(BPC):
            n = slot_counts[s][0]
            b = int(order[perm[s] * NCORES + c])
            fa[s, :, 0:512] = qh[b]
            fa[s, :, 512:512 + n] = bia[b, :, :n]
            fa[s, :, 512 + n:512 + n + 128] = ktd[b, :, 0]
            if n > 1:
                fb[s, :, :128 * (n - 1)] = (
                    ktd[b, :, 1:n].reshape(128, 128 * (n - 1))
                )
            vo = 128 * (n - 1)
            fb[s, :, vo:vo + 66 * n] = vp[b, :, :n, :66].reshape(128, 66 * n)
        in_maps.append({"inpa": fa, "inpb": fb})
    return slot_counts, order, perm, in_maps


def kernel(queries, keys, values, valid_lens):
    slot_counts, order, perm, in_maps = _host_prep(
        queries, keys, values, valid_lens
    )
    nc = _get_nc(slot_counts)
    res = run_bass_kernel_spmd(nc, in_maps, core_ids=list(range(NCORES)))

    out = np.empty((B, S, D), np.float32)
    for c in range(NCORES):
        otv = res.results[c]["ot"]  # [BPC, 128, NQT, D] f16
        ids = [int(order[perm[s] * NCORES + c]) for s in range(BPC)]
        out[ids] = otv.transpose(0, 2, 1, 3).reshape(BPC, S, D).astype(np.float32)
    return out


# revision 13
# speedup vs baseline: 1.1027x; 1.1027x over previous
"""Masked dot-product attention (B=64, S=1024, D=64) on 8 Trainium2 NeuronCores.

Strategy (per core, 8 batches, valid-length-specialized to n k-chunks/batch):
  - Two fused input DMAs per batch: head tile [Qhalf | bias | Kt chunk0]
    (1.3KB/partition, unblocks QK+exp immediately) and bulk tile
    [Kt chunks 1.. | V'] that only gates later chunks and PV.
  - S^T chunks [k=128, q=1024] = K_chunk @ Q^T on PE, D=64 contraction on
    partitions; the two 64-row strips of the PE array compute the two q-halves
    of the SAME chunk concurrently (tile_position row packing, no Q dup).
  - exp is split across TWO engines (ACT was the sole bottleneck at ~37us):
      * ACT chunks: exp via ACT, per-partition bias 0/-30000 folds the
        valid_lens mask into the softmax; 1/sqrt(D) folded into the ACT scale.
      * DVE chunks (fully-valid-for-the-whole-deal-group only): dual-offset
        Schraudolph. One tensor_scalar computes B1 = rint(s*C1 + C2) as int16
        (the f16 exponent-bias add folds into the float affine BEFORE the
        rounding); bitcast f16 gives R1 ~ c*2^y. A second int16 op B2 = B1+512
        gives R2 ~ c*sqrt(2)*2^y. R1+R2 averages the two half-period-offset
        linear-mantissa approximations, cancelling the dominant ripple
        harmonic: +0.6/-1.5% max element error (vs +-4% single), ~2.5e-3
        end-to-end. The add R1+R2 happens for free in PSUM: both are fed as
        separate lhsT slices to the PV accumulation (PE has slack).
  - P @ [V | 1]: P^T slices stationary; column 64 of the accumulator is the
    softmax denominator. normalize = reciprocal + one broadcast
    scalar_tensor_tensor per 4-q-tile half -> fp16 out, upcast on host.
Host does layout prep only (transpose/cast/pack/shard) - all FLOPs on device.
"""

import contextlib

import numpy as np

import concourse.bass as bass  # noqa: F401
import concourse.bacc as bacc
import concourse.mybir as mybir
import concourse.tile as tile
from concourse.bass_utils import run_bass_kernel_spmd

B, S, D = 64, 1024, 64
NCORES = 8
BPC = B // NCORES          # batches per core
NCH = S // 128             # k chunks of 128
NQT = S // 128             # q tiles of 128
F16 = mybir.dt.float16
F32 = mybir.dt.float32
I16 = mybir.dt.int16

# fused input row layout (f16 elements per partition):
#   [0:512)                q half (strip 0: q 0-511, strip 1: q 512-1023)
#   [512 : 512+n)          exp bias per chunk (0 / -30000, f16)
#   [512+n : 512+n+128)    K^T chunk 0
ROW = 512 + 195 * NCH  # legacy single-row length, kept for bench scripts
AROW = 512 + NCH + 128        # [qt | bias(NCH) | kt chunk0]
BROW = 128 * (NCH - 1) + 66 * NCH  # [kt chunks 1.. | vv]

# Dual-offset Schraudolph constants (DVE exp path).
#   B1 = rint(s_raw*SCHR_C1 + SCHR_C2) int16; R1 = bitcast_f16(B1)
#   B2 = B1 + 512;                           R2 = bitcast_f16(B2)
#   R1 + R2 ~ exp(s_raw/8), unbiased (c = 0.398020883 calibrated so the
#   mean ratio over a uniform mantissa phase is exactly 1).
SCHR_C1 = 184.664965234          # 1024 * log2(e) / 8
SCHR_C2 = 13999.018016           # 1024 * (15 + log2(0.398020883))

# Total DVE-offloaded chunks per core (balances ACT ~850ns/chunk vs DVE
# ~1.5us/chunk + ~8.5us finish). Tuned on HW fast-mode observations.
DVE_TARGET = 9

# normalize flavor: one broadcast scalar_tensor_tensor per half (True) vs
# four tensor_scalar_mul per half (False)
NORM_STT = True

# emission order: pace PV behind one unit + emit finish(prev) as soon as
# PV(prev) completes (True), vs baseline order (False). A/B on HW showed
# no benefit; keep the baseline-proven order.
REORDER = False

_NC_CACHE = {}


def _parse_slot(sc):
    if isinstance(sc, tuple):
        if len(sc) == 3:
            n, nz, nd = sc
        else:
            n, nz = sc
            nd = 0
    else:
        n, nz, nd = sc, sc, 0
    n = max(1, min(NCH, n))
    nz = max(1, min(n, nz))
    nd = max(0, min(nd, nz - 1))
    return n, nz, nd


def _interleave(acts, dves):
    # merge the two chunk lists evenly, dve first (chunk 0's kt rides the
    # head tile, so its QK can start before the bulk DMA lands)
    out = []
    na, nd = len(acts), len(dves)
    ia = idv = 0
    for _ in range(na + nd):
        if idv < nd and (ia >= na or idv * na <= ia * nd):
            out.append(("dve", dves[idv]))
            idv += 1
        else:
            out.append(("act", acts[ia]))
            ia += 1
    return out


def _build_nc(loop_reps=None, slot_counts=(NCH,) * BPC, ablate=frozenset(),
              unroll=1):
    # slot_counts entries: n | (n, nz) | (n, nz, nd). Chunks 0..nz-2 are
    # fully valid for EVERY batch dealt into that slot (group min need);
    # chunks 0..nd-1 of those run the DVE dual-Schraudolph exp instead of
    # the ACT exp. The rest always read their per-chunk bias vector on ACT.
    parsed = [_parse_slot(sc) for sc in slot_counts]
    # pm-ring sizing: tiles of slot s (n + nd of them) live through slot s+1
    tiles_per_slot = [n + nd for (n, nz, nd) in parsed]
    max_adj = max(
        (tiles_per_slot[i] + tiles_per_slot[i + 1]
         for i in range(len(tiles_per_slot) - 1)),
        default=tiles_per_slot[0] if tiles_per_slot else 2,
    )
    pm_bufs = min(32, max_adj + 2)

    nc = bacc.Bacc(None, target_bir_lowering=False)
    inpa = nc.dram_tensor("inpa", [BPC, 128, AROW], F16, kind="ExternalInput")
    inpb = nc.dram_tensor("inpb", [BPC, 128, BROW], F16, kind="ExternalInput")
    ot = nc.dram_tensor("ot", [BPC, 128, NQT, D], F16, kind="ExternalOutput")

    with tile.TileContext(nc) as tc:
        with (
            tc.tile_pool(name="inpool", bufs=3) as inpool,
            tc.tile_pool(name="ppool", bufs=pm_bufs) as ppool,
            tc.tile_pool(name="outpool", bufs=2) as outpool,
            tc.tile_pool(name="rpool", bufs=4) as rpool,
            tc.tile_pool(name="spool", bufs=3, space="PSUM") as spool,
            tc.tile_pool(name="accpool", bufs=1, space="PSUM") as accpool,
            tc.For_i(0, loop_reps, 1) if loop_reps else contextlib.nullcontext(),
        ):
            def emit_pv_block(prev, t):
                # one q-tile's full accumulation chain for the previous slot
                b_p, n_p, tb_p, ents_p, acc0_p, acc1_p = prev
                vo_p = 128 * (n_p - 1)
                acc = acc0_p if t < 4 else acc1_p
                total = sum(len(e) for e in ents_p)
                i = 0
                for c in range(n_p):
                    rhs = tb_p[:, vo_p + 66 * c: vo_p + 66 * c + 65]
                    for pm_t, off in ents_p[c]:
                        nc.tensor.matmul(
                            acc[:, t % 4, :],
                            lhsT=pm_t[:, off + t * 128:off + (t + 1) * 128],
                            rhs=rhs,
                            start=(i == 0), stop=(i == total - 1),
                        )
                        i += 1

            def emit_finish_half(prev, half, osb):
                # normalize + store one 4-q-tile half of the previous slot
                b_p, n_p, tb_p, ents_p, acc0_p, acc1_p = prev
                acc = acc0_p if half == 0 else acc1_p
                # denominators are bounded away from 0 (every batch has >=1
                # valid key; DVE chunks are fully valid), so no epsilon guard
                r = rpool.tile([128, 4], F32, tag="r", name="r")
                nc.vector.reciprocal(r, acc[:, :, D])
                if NORM_STT:
                    nc.vector.scalar_tensor_tensor(
                        out=osb[:, half * 4:(half + 1) * 4, :],
                        in0=acc[:, 0:4, 0:D],
                        in1=r.unsqueeze(2).broadcast_to([128, 4, D]),
                        scalar=1.0,
                        op0=mybir.AluOpType.mult, op1=mybir.AluOpType.mult,
                    )
                else:
                    for t4 in range(4):
                        t = half * 4 + t4
                        nc.vector.tensor_scalar_mul(
                            out=osb[:, t, :],
                            in0=acc[:, t4, 0:D],
                            scalar1=r[:, t4:t4 + 1],
                        )
                nc.sync.dma_start(
                    out=ot.ap()[b_p][:, half * 4:(half + 1) * 4, :],
                    in_=osb[:, half * 4:(half + 1) * 4, :],
                )

            def emit_finish(prev):
                osb = outpool.tile([128, NQT, D], F16, name="osb")
                emit_finish_half(prev, 0, osb)
                emit_finish_half(prev, 1, osb)

            # tiny dummy exp: pulls the one-time ~2.7us ACT table load to
            # t=0 so it overlaps the first input DMA instead of serializing
            # before the first real exp
            warm = rpool.tile([128, 1], F32, tag="warm", name="warm")
            nc.vector.memset(warm, 0.0)
            nc.scalar.activation(
                out=warm, in_=warm, func=mybir.ActivationFunctionType.Exp
            )

            prev = None
            for b_u in range(BPC * unroll):
                b = b_u % BPC
                n, nz, nd = parsed[b]
                ua = 512 + n + 128
                ub = 128 * (n - 1) + 66 * n

                ta = inpool.tile([128, AROW], F16, tag="ta", name="ta")
                nc.sync.dma_start(out=ta[:, :ua], in_=inpa.ap()[b][:, :ua])
                tb = inpool.tile([128, BROW], F16, tag="tb", name="tb")
                nc.sync.dma_start(out=tb[:, :ub], in_=inpb.ap()[b][:, :ub])
                qt = ta[:, 0:512]

                acc0 = accpool.tile([128, 4, D + 1], F32, tag="acc0")
                acc1 = accpool.tile([128, 4, D + 1], F32, tag="acc1")

                units = _interleave(list(range(nd, n)), list(range(nd)))

                # interleave this slot's QK+exp with the previous slot's PV so
                # the in-order PE queue never parks ACT behind a PV burst
                ents = [None] * n
                nu = len(units)
                m = max(nu, NQT if prev else 0)
                pv_done = 0
                fin_done = False
                for i in range(m):
                    if prev is not None and "pv" not in ablate:
                        # pace on i (not i+1): the first unit of this slot is
                        # emitted before the first PV block, so the PE never
                        # stalls on finish(prev)'s acc-ring WAR at the slot
                        # boundary
                        ii = i if REORDER else i + 1
                        pv_goal = min(NQT, (NQT * ii + m - 1) // m)
                        while pv_done < pv_goal:
                            emit_pv_block(prev, pv_done)
                            pv_done += 1
                        if REORDER and pv_done == NQT and not fin_done:
                            # emit finish as soon as PV(prev) is complete so
                            # its DVE ops queue ahead of this slot's exp tail
                            emit_finish(prev)
                            fin_done = True
                    if i < nu:
                        kind, c = units[i]
                        st = spool.tile([128, S], F32, tag="st", name="st")
                        if "qk" not in ablate:
                            kt = (
                                ta[:, 512 + n:512 + n + 128] if c == 0
                                else tb[:, 128 * (c - 1):128 * c]
                            )
                            nc.tensor.matmul(
                                st[:, 0:512],
                                lhsT=kt[0:64, :], rhs=qt[0:64, :],
                                start=True, stop=True,
                            )
                            nc.tensor.matmul(
                                st[:, 512:1024],
                                lhsT=kt[64:128, :], rhs=qt[64:128, :],
                                start=True, stop=True,
                            )
                        if "exp" not in ablate:
                            if kind == "act":
                                pm = ppool.tile([128, S], F16, tag="pm",
                                                name="pm")
                                bias = ta[:, 512 + c:512 + c + 1]
                                nc.scalar.activation(
                                    out=pm, in_=st,
                                    func=mybir.ActivationFunctionType.Exp,
                                    bias=bias, scale=0.125,
                                )
                                ents[c] = [(pm, 0)]
                            else:
                                b1 = ppool.tile([128, S], I16, tag="pm",
                                                name="b1")
                                nc.vector.tensor_scalar(
                                    out=b1, in0=st,
                                    scalar1=SCHR_C1, scalar2=SCHR_C2,
                                    op0=mybir.AluOpType.mult,
                                    op1=mybir.AluOpType.add,
                                )
                                b2 = ppool.tile([128, S], I16, tag="pm",
                                                name="b2")
                                nc.vector.tensor_scalar_add(
                                    out=b2, in0=b1, scalar1=512
                                )
                                ents[c] = [(b1.bitcast(F16), 0),
                                           (b2.bitcast(F16), 0)]
                        else:
                            ents[c] = [(tb[:, 0:S].bitcast(F16), 0)]
                if prev is not None and "pv" not in ablate:
                    while pv_done < NQT:
                        emit_pv_block(prev, pv_done)
                        pv_done += 1
                if prev is not None:
                    if "pv" not in ablate:
                        if not fin_done:
                            emit_finish(prev)
                    else:
                        b_p, n_p, tb_p, ents_p = prev[:4]
                        src = ents_p[-1][0][0]
                        nc.sync.dma_start(
                            out=ot.ap()[b_p],
                            in_=src[:, 0:NQT * D].rearrange(
                                "p (t d) -> p t d", d=D
                            ),
                        )
                prev = (b, n, tb, ents, acc0, acc1)

            # drain the last slot: finish+store half 0 while half 1's PV runs
            if "pv" not in ablate:
                osb = outpool.tile([128, NQT, D], F16, name="osb")
                for t in range(NQT):
                    emit_pv_block(prev, t)
                    if t == 3:
                        emit_finish_half(prev, 0, osb)
                emit_finish_half(prev, 1, osb)
            else:
                b_p, n_p, tin_p, ents_p = prev[:4]
                src = ents_p[-1][0][0]
                nc.sync.dma_start(
                    out=ot.ap()[b_p],
                    in_=src[:, 0:NQT * D].rearrange("p (t d) -> p t d", d=D),
                )

    nc.compile()
    return nc


def _get_nc(slot_counts=(NCH,) * BPC):
    key = tuple(slot_counts)
    if key not in _NC_CACHE:
        _NC_CACHE[key] = _build_nc(slot_counts=key)
    return _NC_CACHE[key]


def _host_prep(queries, keys, values, valid_lens):
    queries = np.asarray(queries, dtype=np.float32)
    keys = np.asarray(keys, dtype=np.float32)
    values = np.asarray(values, dtype=np.float32)
    lens = np.asarray(valid_lens).astype(np.int64)

    q16 = queries.astype(np.float16)
    k16 = keys.astype(np.float16)
    v16 = values.astype(np.float16)

    # q halves packed into the two PE row strips: [B, 128, 512]
    qh = q16.transpose(0, 2, 1).reshape(B, 64, 2, 512)
    qh = np.ascontiguousarray(qh.transpose(0, 2, 1, 3)).reshape(B, 128, 512)

    # K^T chunks duplicated into both strips: [B, 128, NCH, 128]
    kt4 = k16.transpose(0, 2, 1).reshape(B, 64, NCH, 128)
    ktd = np.concatenate([kt4, kt4], axis=1)

    # V with ones column (pad to 66): [B, 128, NCH, 66]
    vp = np.zeros((B, 128, NCH, D + 2), np.float16)
    vp[:, :, :, :D] = v16.reshape(B, NCH, 128, D).transpose(0, 2, 1, 3)
    vp[:, :, :, D] = np.float16(1.0)

    # exp bias: 0 where k position valid, -30000 where masked: [B, 128, NCH]
    kpos = np.arange(S).reshape(NCH, 128).T  # [128, NCH] -> k = c*128 + p
    bia = np.where(
        kpos[None] < lens[:, None, None], np.float16(0.0), np.float16(-30000.0)
    ).astype(np.float16)

    # Length specialization: batch i needs ceil(L_i/128) k-chunks (min 1).
    # Sort by need, deal round-robin -> every core's slot s holds batches of
    # (near-)equal need; slot count = max within the deal group, so all cores
    # run the identical compiled program, perfectly balanced.
    need = np.maximum(1, -(-lens // 128)).astype(np.int64)
    order = np.argsort(need, kind="stable")
    gmax = [int(need[order[g * NCORES:(g + 1) * NCORES]].max()) for g in range(BPC)]
    gmin = [int(need[order[g * NCORES:(g + 1) * NCORES]].min()) for g in range(BPC)]
    perm = list(range(BPC - 1, -1, -1))  # descending: smallest slot last = tiny drain tail
    # distribute DVE_TARGET chunks across slots proportional to eligibility
    elig = [max(0, gmin[p] - 1) for p in perm]
    tot_el = sum(elig)
    tgt = min(DVE_TARGET, tot_el)
    nd = [0] * BPC
    if tot_el > 0 and tgt > 0:
        nd = [min(e, (e * tgt) // tot_el) for e in elig]
        rem = tgt - sum(nd)
        frac = sorted(
            range(BPC), key=lambda s: (elig[s] - nd[s]), reverse=True
        )
        for s in frac:
            if rem <= 0:
                break
            if nd[s] < elig[s]:
                nd[s] += 1
                rem -= 1
    slot_counts = tuple(
        (gmax[p], gmin[p], nd[si]) for si, p in enumerate(perm)
    )

    in_maps = []
    for c in range(NCORES):
        fa = np.zeros((BPC, 128, AROW), np.float16)
        fb = np.zeros((BPC, 128, BROW), np.float16)
        for s in range(BPC):
            n = slot_counts[s][0]
            b = int(order[perm[s] * NCORES + c])
            fa[s, :, 0:512] = qh[b]
            fa[s, :, 512:512 + n] = bia[b, :, :n]
            fa[s, :, 512 + n:512 + n + 128] = ktd[b, :, 0]
            if n > 1:
                fb[s, :, :128 * (n - 1)] = (
                    ktd[b, :, 1:n].reshape(128, 128 * (n - 1))
                )
            vo = 128 * (n - 1)
            fb[s, :, vo:vo + 66 * n] = vp[b, :, :n, :66].reshape(128, 66 * n)
        in_maps.append({"inpa": fa, "inpb": fb})
    return slot_counts, order, perm, in_maps


def kernel(queries, keys, values, valid_lens):
    slot_counts, order, perm, in_maps = _host_prep(
        queries, keys, values, valid_lens
    )
    nc = _get_nc(slot_counts)
    res = run_bass_kernel_spmd(nc, in_maps, core_ids=list(range(NCORES)))

    out = np.empty((B, S, D), np.float32)
    for c in range(NCORES):
        otv = res.results[c]["ot"]  # [BPC, 128, NQT, D] f16
        ids = [int(order[perm[s] * NCORES + c]) for s in range(BPC)]
        out[ids] = otv.transpose(0, 2, 1, 3).reshape(BPC, S, D).astype(np.float32)
    return out
